# revision 13
# baseline (speedup 1.0000x reference)
"""Tropical (max-plus) dense layer on 8 Trainium2 NeuronCores.

    out[b, j] = max_i (x[b, i] - W[i, j]) + bias[j],   B = 128, N = 1024.

Strategy (j-sharded SPMD over 8 cores; core c owns j in [c*128, (c+1)*128)):

  The max-reduction cannot run on the TensorEngine and any elementwise
  formulation needs a partition-broadcast, so the PE is used as a
  broadcast + outer-sum machine: for each group of Q=4 reduction indices,
  two accumulating matmuls produce the full term tile
      T[b, (j, q)] = x[b, i_q] - V[i_q, j]        (V = W - bias, host-folded)
  into one PSUM bank ([128, 512]):
    MM_A: stationary = x-column limb rows, moving = constant indicator rows
    MM_B: stationary = constant -1 selector rows, moving = V-row limbs
  (for the offset-0 group of each quarter both halves merge into a single
  K=10 matmul from dedicated xcp0/wvp0 layouts)
  The VectorE max-reduces PSUM (axis=XY over (tile, q)) into per-superblock
  partials, ping-ponging two 4-bank PSUM halves against the PE, then does a
  final reduce over superblocks.

  fp32r (reduced-mantissa fp32, full PE rate) is made accurate by splitting
  x and V into 2 limbs each (hi = round-to-8-bit-mantissa, lo = residual,
  rounded the same way); products against +-1/0 are exact and the fp32 PSUM
  accumulation recovers values to ~2^-18 relative.

  Matmul operand windows must start at 32-aligned partitions, so data rows
  are packed densely inside each 32-partition quarter and the constant-side
  operand carries leading zero rows to null out the other groups' rows that
  fall inside the same window.
"""
import numpy as np

import concourse.bacc as bacc
import concourse.bass as bass
import concourse.mybir as mybir
from concourse.bass_utils import run_bass_kernel_spmd

F32 = mybir.dt.float32
F32R = mybir.dt.float32r

B = 128          # batch (partition dim of output)
N = 1024         # size_in == size_out
NC = 8           # cores
NJ = N // NC     # j-chunk per core = 128
Q = 4            # i's packed per matmul tile
NT = NJ * Q      # matmul free dim = 512
NG = N // Q      # 256 groups (i-blocks of 4)
SB_TILES = 4     # tiles per superblock (= 4 PSUM banks)
NSB = NG // SB_TILES  # 64 superblocks


def _round_m8(a: np.ndarray) -> np.ndarray:
    """Round fp32 to 8-bit stored mantissa, round-half-even — representable
    in any fp32r variant with >= 8 mantissa bits."""
    u = np.ascontiguousarray(a, np.float32).view(np.uint32)
    lsb = (u >> np.uint32(15)) & np.uint32(1)
    r = (u + np.uint32(0x7FFF) + lsb) & np.uint32(0xFFFF8000)
    return r.view(np.float32)


def _split2(a: np.ndarray):
    hi = _round_m8(a)
    lo = _round_m8(a.astype(np.float32) - hi)
    return hi, lo


def _pack_inputs(x: np.ndarray, weights: np.ndarray, bias: np.ndarray):
    """Build the four packed SBUF images.

    xtp  [128, 16*128]: group g=(cb*16 + qt*4 + o): rows qt*32+8o..+8 =
         [xh[:,4g..4g+4].T ; xl rows], cols cb*128..+128.  (shared)
    indv [128, 4*512]:  variant o: cols o*512..+512, rows qt*32..qt*32+8(o+1):
         8o zero rows then [ind0..3, ind0..3], replicated per quarter. (shared)
    negc [128, 4*128]:  variant o: cols o*128..+128, rows qt*32..+2(o+1):
         2*o zero rows then two -1 rows, replicated per quarter. (shared)
    vpk  [128, 16*512]: per core: group g=(cb*16 + qt*4 + o): rows
         qt*32+2o..+2 = [Vh-pack(g); Vl-pack(g)], cols cb*512..+512.
         (same quarter as the group's x-rows so MM_A/MM_B share tile_position
         — HW faults on accumulating matmuls with different row-groups)
    """
    xh, xl = _split2(x)                                  # [B, N]
    V = weights.astype(np.float32) - bias.astype(np.float32)[None, :]
    Vh, Vl = _split2(V)                                  # [N, N]

    xtp = np.zeros((128, 16 * 128), np.float32)
    xhT, xlT = xh.T, xl.T
    for g in range(NG):
        cb, r = divmod(g, 16)
        qt, o = divmod(r, 4)
        p0, c0 = qt * 32 + 8 * o, cb * 128
        xtp[p0:p0 + 4, c0:c0 + 128] = xhT[4 * g:4 * g + 4]
        xtp[p0 + 4:p0 + 8, c0:c0 + 128] = xlT[4 * g:4 * g + 4]

    n_idx = np.arange(NT)
    ind = (n_idx[None, :] % Q == np.arange(Q)[:, None]).astype(np.float32)  # [4,512]
    ind8 = np.concatenate([ind, ind], axis=0)            # [8, 512]

    indv = np.zeros((128, 3 * NT), np.float32)
    negc = np.zeros((128, 3 * 128), np.float32)
    for qt in range(4):
        for o in range(1, 4):
            indv[qt * 32 + 8 * o:qt * 32 + 8 * o + 8, (o - 1) * NT:o * NT] = ind8
            negc[qt * 32 + 2 * o:qt * 32 + 2 * o + 2, (o - 1) * 128:o * 128] = -1.0

    # merged single-matmul operands for the o==0 group of each (cb, qt):
    # lhsT rows [xh4; xl4; -1; -1], rhs rows [ind8; Vh; Vl]
    xcp0 = np.zeros((128, 16 * 128), np.float32)
    for cb in range(16):
        for qt in range(4):
            g = cb * 16 + qt * 4
            p0, c0 = qt * 32, cb * 128
            xcp0[p0:p0 + 4, c0:c0 + 128] = xhT[4 * g:4 * g + 4]
            xcp0[p0 + 4:p0 + 8, c0:c0 + 128] = xlT[4 * g:4 * g + 4]
            xcp0[p0 + 8:p0 + 10, c0:c0 + 128] = -1.0

    wpks = []
    wvp0s = []
    for c in range(NC):
        jc = c * NJ
        vpk = np.zeros((128, 16 * NT), np.float32)
        wvp0 = np.zeros((128, 16 * NT), np.float32)
        for g in range(NG):
            cbv, r = divmod(g, 16)
            qtv, ov = divmod(r, 4)
            p0, c0 = qtv * 32 + 2 * ov, cbv * NT
            vh = Vh[4 * g:4 * g + 4, jc:jc + NJ].T.reshape(-1)
            vl = Vl[4 * g:4 * g + 4, jc:jc + NJ].T.reshape(-1)
            if ov == 0:
                wvp0[qtv * 32:qtv * 32 + 8, c0:c0 + NT] = ind8
                wvp0[qtv * 32 + 8, c0:c0 + NT] = vh
                wvp0[qtv * 32 + 9, c0:c0 + NT] = vl
            else:
                vpk[p0, c0:c0 + NT] = vh
                vpk[p0 + 1, c0:c0 + NT] = vl
        wpks.append(vpk)
        wvp0s.append(wvp0)
    return xtp, indv, negc, xcp0, wpks, wvp0s


def _build_program() -> bass.Bass:
    nc = bacc.Bacc("TRN2", target_bir_lowering=False, debug=False)

    xtp_d = nc.dram_tensor("xtp", [128, 16 * 128], F32R, kind="ExternalInput")
    indv_d = nc.dram_tensor("indv", [128, 3 * NT], F32R, kind="ExternalInput")
    negc_d = nc.dram_tensor("negc", [128, 3 * 128], F32R, kind="ExternalInput")
    vpk_d = nc.dram_tensor("vpk", [128, 16 * NT], F32R, kind="ExternalInput")
    xcp0_d = nc.dram_tensor("xcp0", [128, 16 * 128], F32R, kind="ExternalInput")
    wvp0_d = nc.dram_tensor("wvp0", [128, 16 * NT], F32R, kind="ExternalInput")
    out_d = nc.dram_tensor("out", [B, NJ], F32, kind="ExternalOutput")

    xtp_s = nc.alloc_sbuf_tensor("xtp_s", [128, 16 * 128], F32R)
    indv_s = nc.alloc_sbuf_tensor("indv_s", [128, 3 * NT], F32R)
    negc_s = nc.alloc_sbuf_tensor("negc_s", [128, 3 * 128], F32R)
    vpk_s = nc.alloc_sbuf_tensor("vpk_s", [128, 16 * NT], F32R)
    xcp0_s = nc.alloc_sbuf_tensor("xcp0_s", [128, 16 * 128], F32R)
    wvp0_s = nc.alloc_sbuf_tensor("wvp0_s", [128, 16 * NT], F32R)
    partials = nc.alloc_sbuf_tensor("partials", [128, NJ, NSB], F32)
    out_s = nc.alloc_sbuf_tensor("out_s", [B, NJ], F32)

    ps = [
        nc.alloc_psum_tensor("ps0", [128, SB_TILES * NT], F32),
        nc.alloc_psum_tensor("ps1", [128, SB_TILES * NT], F32),
    ]

    const_sem = nc.alloc_semaphore("const_sem")
    ch_sems = [nc.alloc_semaphore(f"ch_sem{c}") for c in range(16)]
    pe_sem = nc.alloc_semaphore("pe_sem")
    dve_sem = nc.alloc_semaphore("dve_sem")
    out_sem = nc.alloc_semaphore("out_sem")

    # constants first, then per-column-block-quad chunks of xtp+vpk so the
    # PE can start after ~1.5MB instead of the full 6.5MB
    nc.sync.dma_start(indv_s[:], indv_d[:]).then_inc(const_sem, 16)
    nc.sync.dma_start(negc_s[:], negc_d[:]).then_inc(const_sem, 16)
    for ch in range(16):
        xs = slice(ch * 128, (ch + 1) * 128)
        vs = slice(ch * NT, (ch + 1) * NT)
        nc.sync.dma_start(xtp_s[:, xs], xtp_d[:, xs]).then_inc(ch_sems[ch], 16)
        nc.sync.dma_start(vpk_s[:, vs], vpk_d[:, vs]).then_inc(ch_sems[ch], 16)
        nc.sync.dma_start(xcp0_s[:, xs], xcp0_d[:, xs]).then_inc(ch_sems[ch], 16)
        nc.sync.dma_start(wvp0_s[:, vs], wvp0_d[:, vs]).then_inc(ch_sems[ch], 16)
        # serialize chunk issuance so early chunks get full DMA bandwidth
        # (eager issue would fair-share and delay chunk 0)
        if ch < 15:
            nc.sync.wait_ge(ch_sems[ch], 64)

    nc.tensor.wait_ge(const_sem, 32)
    for sb in range(NSB):
        pp = ps[sb & 1]
        if sb % 4 == 0:
            # chunk sb//4 (= column block cb) must have landed
            nc.tensor.wait_ge(ch_sems[sb // 4], 64)
        if sb >= 2:
            nc.tensor.wait_ge(dve_sem, sb - 1)  # DVE done with superblock sb-2
        mm = None
        for t in range(SB_TILES):
            g = sb * SB_TILES + t
            bank = pp[:, t * NT:(t + 1) * NT]
            cb, r = divmod(g, 16)
            qt, o = divmod(r, 4)
            if o == 0:
                # offset-0 window has no garbage rows: single merged matmul
                mm = nc.tensor.matmul(
                    bank,
                    lhsT=xcp0_s[qt * 32:qt * 32 + 10, cb * 128:(cb + 1) * 128],
                    rhs=wvp0_s[qt * 32:qt * 32 + 10, cb * NT:(cb + 1) * NT],
                    start=True, stop=True, tile_position=(qt * 32, 0),
                )
            else:
                nc.tensor.matmul(
                    bank,
                    lhsT=xtp_s[qt * 32:qt * 32 + 8 * (o + 1), cb * 128:(cb + 1) * 128],
                    rhs=indv_s[qt * 32:qt * 32 + 8 * (o + 1),
                               (o - 1) * NT:o * NT],
                    start=True, stop=False, tile_position=(qt * 32, 0),
                )
                mm = nc.tensor.matmul(
                    bank,
                    lhsT=negc_s[qt * 32:qt * 32 + 2 * (o + 1),
                                (o - 1) * 128:o * 128],
                    rhs=vpk_s[qt * 32:qt * 32 + 2 * (o + 1),
                              cb * NT:(cb + 1) * NT],
                    start=False, stop=True, tile_position=(qt * 32, 0),
                )
        mm.then_inc(pe_sem, 1)

    for sb in range(NSB):
        pp = ps[sb & 1]
        nc.vector.wait_ge(pe_sem, sb + 1)
        red_in = pp[:].rearrange("p (t j q) -> p j t q", t=SB_TILES, q=Q)
        nc.vector.tensor_reduce(
            out=partials[:, :, sb], in_=red_in,
            axis=mybir.AxisListType.XY, op=mybir.AluOpType.max,
        ).then_inc(dve_sem, 1)

    nc.vector.wait_ge(dve_sem, NSB)
    nc.vector.tensor_reduce(
        out=out_s[:], in_=partials[:],
        axis=mybir.AxisListType.X, op=mybir.AluOpType.max,
    ).then_inc(dve_sem, 1)

    nc.sync.wait_ge(dve_sem, NSB + 1)
    nc.sync.dma_start(out_d[:], out_s[:]).then_inc(out_sem, 16)
    nc.sync.wait_ge(out_sem, 16)
    nc.compile()
    return nc


_nc_cache = None


def _get_nc():
    global _nc_cache
    if _nc_cache is None:
        _nc_cache = _build_program()
    return _nc_cache


def kernel(x: np.ndarray, weights: np.ndarray, bias: np.ndarray, _trace=False):
    x = np.asarray(x, np.float32)
    weights = np.asarray(weights, np.float32)
    bias = np.asarray(bias, np.float32)

    xtp, indv, negc, xcp0, wpks, wvp0s = _pack_inputs(x, weights, bias)
    in_maps = [
        {"xtp": xtp, "indv": indv, "negc": negc, "xcp0": xcp0,
         "vpk": wpks[c], "wvp0": wvp0s[c]}
        for c in range(NC)
    ]

    nc = _get_nc()
    res = run_bass_kernel_spmd(nc, in_maps, core_ids=list(range(NC)), trace=_trace)
    out = np.concatenate([res.results[c]["out"] for c in range(NC)], axis=1)
    if _trace:
        return out, res
    return out


if __name__ == "__main__":
    rng = np.random.default_rng(0)
    x = rng.standard_normal((B, N)).astype(np.float32)
    w = rng.standard_normal((N, N)).astype(np.float32)
    b = rng.standard_normal(N).astype(np.float32)
    got = kernel(x, w, b)
    exp = (x[:, :, None] - w).max(axis=1) + b
    d = np.abs(got - exp)
    rel = d / (np.abs(exp) + 1e-9)
    print(f"maxabs={d.max():.3e} maxrel={rel.max():.3e}")


# revision 18
# speedup vs baseline: 1.0406x; 1.0406x over previous
"""Tropical (max-plus) dense layer on 8 Trainium2 NeuronCores.

    out[b, j] = max_i (x[b, i] - W[i, j]) + bias[j],   B = 128, N = 1024.

Strategy (j-sharded SPMD over 8 cores; core c owns j in [c*128, (c+1)*128)):

  The max-reduction cannot run on the TensorEngine and any elementwise
  formulation needs a partition-broadcast, so the PE is used as a
  broadcast + outer-sum machine: for each group of Q=4 reduction indices,
  two accumulating matmuls produce the full term tile
      T[b, (j, q)] = x[b, i_q] - V[i_q, j]        (V = W - bias, host-folded)
  into one PSUM bank ([128, 512]):
    MM_A: stationary = x-column limb rows, moving = constant indicator rows
    MM_B: stationary = constant -1 selector rows, moving = V-row limbs
  (for the offset-0 group of each quarter both halves merge into a single
  K=10 matmul from dedicated xcp0/wvp0 layouts)
  The VectorE max-reduces PSUM (axis=XY over (tile, q)) into per-superblock
  partials, ping-ponging two 4-bank PSUM halves against the PE, then does a
  final reduce over superblocks.

  fp32r (reduced-mantissa fp32, full PE rate) is made accurate by splitting
  x and V into 2 limbs each (hi = round-to-8-bit-mantissa, lo = residual,
  rounded the same way); products against +-1/0 are exact and the fp32 PSUM
  accumulation recovers values to ~2^-18 relative.

  Matmul operand windows must start at 32-aligned partitions, so data rows
  are packed densely inside each 32-partition quarter and the constant-side
  operand carries leading zero rows to null out the other groups' rows that
  fall inside the same window.
"""
import numpy as np

import concourse.bacc as bacc
import concourse.bass as bass
import concourse.mybir as mybir
from concourse.bass_utils import run_bass_kernel_spmd

F32 = mybir.dt.float32
F32R = mybir.dt.float32r

B = 128          # batch (partition dim of output)
N = 1024         # size_in == size_out
NC = 8           # cores
NJ = N // NC     # j-chunk per core = 128
Q = 4            # i's packed per matmul tile
NT = NJ * Q      # matmul free dim = 512
NG = N // Q      # 256 groups (i-blocks of 4)
SB_TILES = 4     # tiles per superblock (= 4 PSUM banks)
NSB = NG // SB_TILES  # 64 superblocks


def _round_m8(a: np.ndarray) -> np.ndarray:
    """Round fp32 to 8-bit stored mantissa, round-half-even — representable
    in any fp32r variant with >= 8 mantissa bits."""
    u = np.ascontiguousarray(a, np.float32).view(np.uint32)
    lsb = (u >> np.uint32(15)) & np.uint32(1)
    r = (u + np.uint32(0x7FFF) + lsb) & np.uint32(0xFFFF8000)
    return r.view(np.float32)


def _split2(a: np.ndarray):
    hi = _round_m8(a)
    lo = _round_m8(a.astype(np.float32) - hi)
    return hi, lo


def _pack_inputs(x: np.ndarray, weights: np.ndarray, bias: np.ndarray):
    """Build the four packed SBUF images.

    xtp  [128, 16*128]: group g=(cb*16 + qt*4 + o): rows qt*32+8o..+8 =
         [xh[:,4g..4g+4].T ; xl rows], cols cb*128..+128.  (shared)
    indv [128, 4*512]:  variant o: cols o*512..+512, rows qt*32..qt*32+8(o+1):
         8o zero rows then [ind0..3, ind0..3], replicated per quarter. (shared)
    negc [128, 4*128]:  variant o: cols o*128..+128, rows qt*32..+2(o+1):
         2*o zero rows then two -1 rows, replicated per quarter. (shared)
    vpk  [128, 16*512]: per core: group g=(cb*16 + qt*4 + o): rows
         qt*32+2o..+2 = [Vh-pack(g); Vl-pack(g)], cols cb*512..+512.
         (same quarter as the group's x-rows so MM_A/MM_B share tile_position
         — HW faults on accumulating matmuls with different row-groups)
    """
    xh, xl = _split2(x)                                  # [B, N]
    V = weights.astype(np.float32) - bias.astype(np.float32)[None, :]
    Vh, Vl = _split2(V)                                  # [N, N]

    xtp = np.zeros((128, 16 * 128), np.float32)
    xhT, xlT = xh.T, xl.T
    for g in range(NG):
        cb, r = divmod(g, 16)
        qt, o = divmod(r, 4)
        p0, c0 = qt * 32 + 8 * o, cb * 128
        xtp[p0:p0 + 4, c0:c0 + 128] = xhT[4 * g:4 * g + 4]
        xtp[p0 + 4:p0 + 8, c0:c0 + 128] = xlT[4 * g:4 * g + 4]

    n_idx = np.arange(NT)
    ind = (n_idx[None, :] % Q == np.arange(Q)[:, None]).astype(np.float32)  # [4,512]
    ind8 = np.concatenate([ind, ind], axis=0)            # [8, 512]

    indv = np.zeros((128, 3 * NT), np.float32)
    negc = np.zeros((128, 3 * 128), np.float32)
    for qt in range(4):
        for o in range(1, 4):
            indv[qt * 32 + 8 * o:qt * 32 + 8 * o + 8, (o - 1) * NT:o * NT] = ind8
            negc[qt * 32 + 2 * o:qt * 32 + 2 * o + 2, (o - 1) * 128:o * 128] = -1.0

    # merged single-matmul operands for the o==0 group of each (cb, qt):
    # lhsT rows [xh4; xl4; -1; -1], rhs rows [ind8; Vh; Vl]
    xcp0 = np.zeros((128, 16 * 128), np.float32)
    for cb in range(16):
        for qt in range(4):
            g = cb * 16 + qt * 4
            p0, c0 = qt * 32, cb * 128
            xcp0[p0:p0 + 4, c0:c0 + 128] = xhT[4 * g:4 * g + 4]
            xcp0[p0 + 4:p0 + 8, c0:c0 + 128] = xlT[4 * g:4 * g + 4]
            xcp0[p0 + 8:p0 + 10, c0:c0 + 128] = -1.0

    wpks = []
    wvp0s = []
    for c in range(NC):
        jc = c * NJ
        vpk = np.zeros((128, 16 * NT), np.float32)
        wvp0 = np.zeros((128, 16 * NT), np.float32)
        for g in range(NG):
            cbv, r = divmod(g, 16)
            qtv, ov = divmod(r, 4)
            p0, c0 = qtv * 32 + 2 * ov, cbv * NT
            vh = Vh[4 * g:4 * g + 4, jc:jc + NJ].T.reshape(-1)
            vl = Vl[4 * g:4 * g + 4, jc:jc + NJ].T.reshape(-1)
            if ov == 0:
                wvp0[qtv * 32:qtv * 32 + 8, c0:c0 + NT] = ind8
                wvp0[qtv * 32 + 8, c0:c0 + NT] = vh
                wvp0[qtv * 32 + 9, c0:c0 + NT] = vl
            else:
                vpk[p0, c0:c0 + NT] = vh
                vpk[p0 + 1, c0:c0 + NT] = vl
        wpks.append(vpk)
        wvp0s.append(wvp0)
    return xtp, indv, negc, xcp0, wpks, wvp0s


def _build_program() -> bass.Bass:
    nc = bacc.Bacc("TRN2", target_bir_lowering=False, debug=False)

    xtp_d = nc.dram_tensor("xtp", [128, 16 * 128], F32R, kind="ExternalInput")
    indv_d = nc.dram_tensor("indv", [128, 3 * NT], F32R, kind="ExternalInput")
    negc_d = nc.dram_tensor("negc", [128, 3 * 128], F32R, kind="ExternalInput")
    vpk_d = nc.dram_tensor("vpk", [128, 16 * NT], F32R, kind="ExternalInput")
    xcp0_d = nc.dram_tensor("xcp0", [128, 16 * 128], F32R, kind="ExternalInput")
    wvp0_d = nc.dram_tensor("wvp0", [128, 16 * NT], F32R, kind="ExternalInput")
    out_d = nc.dram_tensor("out", [B, NJ], F32, kind="ExternalOutput")

    xtp_s = nc.alloc_sbuf_tensor("xtp_s", [128, 16 * 128], F32R)
    indv_s = nc.alloc_sbuf_tensor("indv_s", [128, 3 * NT], F32R)
    negc_s = nc.alloc_sbuf_tensor("negc_s", [128, 3 * 128], F32R)
    vpk_s = nc.alloc_sbuf_tensor("vpk_s", [128, 16 * NT], F32R)
    xcp0_s = nc.alloc_sbuf_tensor("xcp0_s", [128, 16 * 128], F32R)
    wvp0_s = nc.alloc_sbuf_tensor("wvp0_s", [128, 16 * NT], F32R)
    partials = nc.alloc_sbuf_tensor("partials", [128, NJ, NSB], F32)
    out_s = nc.alloc_sbuf_tensor("out_s", [B, NJ], F32)

    ps = [
        nc.alloc_psum_tensor("ps0", [128, SB_TILES * NT], F32),
        nc.alloc_psum_tensor("ps1", [128, SB_TILES * NT], F32),
    ]

    fast_sem = nc.alloc_semaphore("fast_sem")
    const_sem = nc.alloc_semaphore("const_sem")
    ch_sems = [nc.alloc_semaphore(f"ch_sem{c}") for c in range(16)]
    pe_sem = nc.alloc_semaphore("pe_sem")
    dve_sem = nc.alloc_semaphore("dve_sem")
    out_sem = nc.alloc_semaphore("out_sem")

    # constants first, then per-column-block-quad chunks of xtp+vpk so the
    # PE can start after ~1.5MB instead of the full 6.5MB
    # fast chunk: only quarter-0 partition rows of what superblock 0 touches
    # (~0.4MB) so the PE can start almost immediately
    for td, ts_, cols in ((indv_d, indv_s, 3 * NT), (negc_d, negc_s, 3 * 128),
                          (xtp_d, xtp_s, 128), (vpk_d, vpk_s, NT),
                          (xcp0_d, xcp0_s, 128), (wvp0_d, wvp0_s, NT)):
        nc.sync.dma_start(ts_[0:32, 0:cols], td[0:32, 0:cols]).then_inc(fast_sem, 16)
    nc.sync.wait_ge(fast_sem, 6 * 16)
    nc.sync.dma_start(indv_s[32:128, :], indv_d[32:128, :]).then_inc(const_sem, 16)
    nc.sync.dma_start(negc_s[32:128, :], negc_d[32:128, :]).then_inc(const_sem, 16)
    for ch in range(16):
        xs = slice(ch * 128, (ch + 1) * 128)
        vs = slice(ch * NT, (ch + 1) * NT)
        p0 = 32 if ch == 0 else 0  # ch0 quarter-0 rows already in the fast chunk
        nc.sync.dma_start(xtp_s[p0:128, xs], xtp_d[p0:128, xs]).then_inc(ch_sems[ch], 16)
        nc.sync.dma_start(vpk_s[p0:128, vs], vpk_d[p0:128, vs]).then_inc(ch_sems[ch], 16)
        nc.sync.dma_start(xcp0_s[p0:128, xs], xcp0_d[p0:128, xs]).then_inc(ch_sems[ch], 16)
        nc.sync.dma_start(wvp0_s[p0:128, vs], wvp0_d[p0:128, vs]).then_inc(ch_sems[ch], 16)
        # serialize chunk issuance so early chunks get full DMA bandwidth
        # (eager issue would fair-share and delay chunk 0)
        if ch < 15:
            nc.sync.wait_ge(ch_sems[ch], 64)

    nc.tensor.wait_ge(fast_sem, 6 * 16)
    for sb in range(NSB):
        pp = ps[sb & 1]
        if sb == 1:
            # rest of the constants + full chunk 0 (quarters 1-3 of cb 0)
            nc.tensor.wait_ge(const_sem, 32)
            nc.tensor.wait_ge(ch_sems[0], 64)
        if sb % 4 == 0 and sb > 0:
            nc.tensor.wait_ge(ch_sems[sb // 4], 64)
        if sb >= 2:
            nc.tensor.wait_ge(dve_sem, sb - 1)  # DVE done with superblock sb-2
        mm = None
        for t in range(SB_TILES):
            g = sb * SB_TILES + t
            bank = pp[:, t * NT:(t + 1) * NT]
            cb, r = divmod(g, 16)
            qt, o = divmod(r, 4)
            if o == 0:
                # offset-0 window has no garbage rows: single merged matmul
                mm = nc.tensor.matmul(
                    bank,
                    lhsT=xcp0_s[qt * 32:qt * 32 + 10, cb * 128:(cb + 1) * 128],
                    rhs=wvp0_s[qt * 32:qt * 32 + 10, cb * NT:(cb + 1) * NT],
                    start=True, stop=True, tile_position=(qt * 32, 0),
                )
            else:
                nc.tensor.matmul(
                    bank,
                    lhsT=xtp_s[qt * 32:qt * 32 + 8 * (o + 1), cb * 128:(cb + 1) * 128],
                    rhs=indv_s[qt * 32:qt * 32 + 8 * (o + 1),
                               (o - 1) * NT:o * NT],
                    start=True, stop=False, tile_position=(qt * 32, 0),
                )
                mm = nc.tensor.matmul(
                    bank,
                    lhsT=negc_s[qt * 32:qt * 32 + 2 * (o + 1),
                                (o - 1) * 128:o * 128],
                    rhs=vpk_s[qt * 32:qt * 32 + 2 * (o + 1),
                              cb * NT:(cb + 1) * NT],
                    start=False, stop=True, tile_position=(qt * 32, 0),
                )
        mm.then_inc(pe_sem, 1)

    for sb in range(NSB):
        pp = ps[sb & 1]
        nc.vector.wait_ge(pe_sem, sb + 1)
        red_in = pp[:].rearrange("p (t j q) -> p j t q", t=SB_TILES, q=Q)
        nc.vector.tensor_reduce(
            out=partials[:, :, sb], in_=red_in,
            axis=mybir.AxisListType.XY, op=mybir.AluOpType.max,
        ).then_inc(dve_sem, 1)

    nc.vector.wait_ge(dve_sem, NSB)
    nc.vector.tensor_reduce(
        out=out_s[:], in_=partials[:],
        axis=mybir.AxisListType.X, op=mybir.AluOpType.max,
    ).then_inc(dve_sem, 1)

    nc.sync.wait_ge(dve_sem, NSB + 1)
    nc.sync.dma_start(out_d[:], out_s[:]).then_inc(out_sem, 16)
    nc.sync.wait_ge(out_sem, 16)
    nc.compile()
    return nc


_nc_cache = None


def _get_nc():
    global _nc_cache
    if _nc_cache is None:
        _nc_cache = _build_program()
    return _nc_cache


def kernel(x: np.ndarray, weights: np.ndarray, bias: np.ndarray, _trace=False):
    x = np.asarray(x, np.float32)
    weights = np.asarray(weights, np.float32)
    bias = np.asarray(bias, np.float32)

    xtp, indv, negc, xcp0, wpks, wvp0s = _pack_inputs(x, weights, bias)
    in_maps = [
        {"xtp": xtp, "indv": indv, "negc": negc, "xcp0": xcp0,
         "vpk": wpks[c], "wvp0": wvp0s[c]}
        for c in range(NC)
    ]

    nc = _get_nc()
    res = run_bass_kernel_spmd(nc, in_maps, core_ids=list(range(NC)), trace=_trace)
    out = np.concatenate([res.results[c]["out"] for c in range(NC)], axis=1)
    if _trace:
        return out, res
    return out


if __name__ == "__main__":
    rng = np.random.default_rng(0)
    x = rng.standard_normal((B, N)).astype(np.float32)
    w = rng.standard_normal((N, N)).astype(np.float32)
    b = rng.standard_normal(N).astype(np.float32)
    got = kernel(x, w, b)
    exp = (x[:, :, None] - w).max(axis=1) + b
    d = np.abs(got - exp)
    rel = d / (np.abs(exp) + 1e-9)
    print(f"maxabs={d.max():.3e} maxrel={rel.max():.3e}")
